# revision 3
# baseline (speedup 1.0000x reference)
"""MoE layer (8 experts, top-2, SwiGLU) on 8 TRN2 NeuronCores.

Strategy: expert-parallel. The router (x @ Wr, top-2, softmax) runs on the
host — it is ~0.03% of the FLOPs. Tokens are dispatched per expert on the
host (the "all-to-all"), each core runs its expert's dense SwiGLU MLP over
its (capacity-padded) token batch, and the host applies the combine weights
(including DEPTH_SCALE) on the way back.

All matmul operands are bf16: on TRN2 the PE runs bf16 at the same
1 row/cycle as fp32r (measured), so bf16 keeps the same PE throughput while
halving DMA traffic and SBUF footprint. PSUM accumulation stays fp32; the
end-to-end relative error is ~4e-3. The halved footprint lets the whole
hidden dimension stay SBUF-resident: a single down-projection phase with
32-chunk accumulation chains and one output tensor.

Device layout (per core / expert e):
  xt   [C, CAP]            gathered tokens, transposed (feature-major), bf16
  wg_t [H/128,128,C/128,128]  Wg[e] pre-tiled so each stationary DMA is
  wu_t same                   contiguous, bf16
  wd_t [C/128,128,H/128,128]  Wd[e] pre-tiled, bf16
  yt   [C, CAP]            (silu(x@Wg) * (x@Wu)) @ Wd, transposed, fp32,
                           unscaled (host applies combine * DEPTH_SCALE)

The kernel is PE-bound: the ablated matmul-only instruction stream measures
the same wall time as the full kernel, i.e. weight streaming, SwiGLU and
output copies are fully hidden behind the matmul chain.
"""

import sys

if "/opt/trn_rl_repo" not in sys.path:
    sys.path.insert(0, "/opt/trn_rl_repo")

import numpy as np
import ml_dtypes

D_MODEL = 1024
HIDDEN = 4096
NUM_EXPERTS = 8
TOP_K = 2
DEPTH_SCALE = 1.0 / np.sqrt(12.0)

P = 128
NC = D_MODEL // P     # 8 column chunks of the model dim
NH = HIDDEN // P      # 32 chunks of the hidden dim

BF16 = ml_dtypes.bfloat16


def _dedup_ldweights_json(bir):
    """Remove redundant PE Ldweights: walrus pairs every MMUL with its own
    LDW (and --enable-ldw-opt is both hardcoded off and incompatible with
    bass BIR), so 3 consecutive matmuls on the same stationary tile reload
    it 3 times, ~27ns of serialized PE time each. Drop an Ldweights whose
    operands exactly match the previous Ldweights with no intervening
    PE-state-changing instruction; waits migrate to the next instruction."""
    import json as _json

    n_removed = 0
    for fn in bir["functions"]:
        for blk in fn["blocks"]:
            out = []
            last_key = None
            pending_waits = []
            for inst in blk["instructions"]:
                if inst.get("engine") != "PE":
                    out.append(inst)
                    continue
                op = inst.get("opcode")
                if op == "Ldweights":
                    key = _json.dumps(
                        [inst.get("ins"), inst.get("tile_position"),
                         inst.get("tile_size"), inst.get("perf_mode"),
                         inst.get("is_transpose")], sort_keys=True)
                    sync = inst.get("sync_info") or {}
                    if (key == last_key and not sync.get("on_update")):
                        n_removed += 1
                        pending_waits.extend(sync.get("on_wait") or [])
                        continue
                    last_key = key
                elif op != "Matmult":
                    last_key = None
                if pending_waits:
                    sync = inst.setdefault(
                        "sync_info", {"on_update": [], "on_wait": []})
                    sync.setdefault("on_wait", []).extend(pending_waits)
                    pending_waits = []
                out.append(inst)
            assert not pending_waits, "dangling waits from removed Ldweights"
            blk["instructions"] = out
    return n_removed


def _patch_ldw_dedup(nc):
    import json as _json

    orig = nc.to_json_bytes

    def patched():
        bir = _json.loads(orig())
        _dedup_ldweights_json(bir)
        return _json.dumps(bir).encode()

    nc.to_json_bytes = patched
    return nc


def _token_subtiles(cap):
    """Pad cap so it splits into equal even-width sub-tiles in [256, 512]
    (full PE rate needs a wide moving free dim; the PSUM bank caps fp32
    matmul width at 512). Returns (padded_cap, subs)."""
    cap = max(256, cap)
    nt = max(1, -(-cap // 512))
    step = 2 * nt
    cap = -(-cap // step) * step
    size = cap // nt
    assert 256 <= size <= 512 and size % 2 == 0, (cap, size)
    return cap, [(i * size, size) for i in range(nt)]


def _build_nc(cap, repeat=1, wgu_bufs=3, wd_bufs=2, y_bufs=2):
    import concourse.mybir as mybir
    import concourse.tile as tile
    from concourse import bacc

    F32 = mybir.dt.float32
    BF = mybir.dt.bfloat16
    cap2, subs = _token_subtiles(cap)
    assert cap2 == cap, (cap2, cap)
    nt = len(subs)
    ps_w = max(s for _, s in subs)

    nc = bacc.Bacc("TRN2", target_bir_lowering=False, debug=False, num_devices=8)
    xt = nc.dram_tensor("xt", [D_MODEL, cap], BF, kind="ExternalInput").ap()
    wg_t = nc.dram_tensor("wg_t", [NH, P, NC, P], BF, kind="ExternalInput").ap()
    wu_t = nc.dram_tensor("wu_t", [NH, P, NC, P], BF, kind="ExternalInput").ap()
    wd_t = nc.dram_tensor("wd_t", [NC, P, NH, P], BF, kind="ExternalInput").ap()
    yt = nc.dram_tensor("yt", [D_MODEL, cap], F32, kind="ExternalOutput").ap()

    from contextlib import ExitStack

    with tile.TileContext(nc) as tc:
        rep = ExitStack()
        if repeat > 1:
            rep.enter_context(tc.For_i(0, repeat, 1))
        with (
            rep,
            tc.tile_pool(name="xpool", bufs=1) as xpool,
            tc.tile_pool(name="hpool", bufs=1) as hpool,
            tc.tile_pool(name="wg", bufs=wgu_bufs) as wgp,
            tc.tile_pool(name="wu", bufs=wgu_bufs) as wup,
            tc.tile_pool(name="wd", bufs=wd_bufs) as wdp,
            tc.tile_pool(name="ypool", bufs=y_bufs) as ypool,
            tc.tile_pool(name="psum", bufs=8, space="PSUM") as psp,
        ):
            # token activations, split per c-chunk so the first gate chain
            # only waits on chunk 0; the first weight tiles are queued right
            # after chunk 0 so the PE can start before the full xt lands
            xt_sb = xpool.tile([P, NC, cap], BF)
            xt_r = xt.rearrange("(o p) n -> p o n", p=P)
            nc.sync.dma_start(xt_sb[:, 0], xt_r[:, 0])
            wg0_sb = wgp.tile([P, NC, P], BF, tag="wg")
            wu0_sb = wup.tile([P, NC, P], BF, tag="wu")
            nc.sync.dma_start(wg0_sb[:], wg_t[0])
            nc.sync.dma_start(wu0_sb[:], wu_t[0])
            for c in range(1, NC):
                nc.sync.dma_start(xt_sb[:, c], xt_r[:, c])

            h_sb = hpool.tile([P, NH, cap], BF, tag="h")
            for hc in range(NH):
                if hc == 0:
                    wg_sb, wu_sb = wg0_sb, wu0_sb
                else:
                    wg_sb = wgp.tile([P, NC, P], BF, tag="wg")
                    wu_sb = wup.tile([P, NC, P], BF, tag="wu")
                    nc.sync.dma_start(wg_sb[:], wg_t[hc])
                    nc.sync.dma_start(wu_sb[:], wu_t[hc])

                pg = [psp.tile([P, ps_w], F32, tag="ps", name=f"pg{hc}_{t}")
                      for t in range(nt)]
                pu = [psp.tile([P, ps_w], F32, tag="ps", name=f"pu{hc}_{t}")
                      for t in range(nt)]
                for c in range(NC):
                    for t, (o, w) in enumerate(subs):
                        nc.tensor.matmul(
                            pg[t][:, :w], wg_sb[:, c], xt_sb[:, c, o:o + w],
                            start=(c == 0), stop=(c == NC - 1),
                        )
                for c in range(NC):
                    for t, (o, w) in enumerate(subs):
                        nc.tensor.matmul(
                            pu[t][:, :w], wu_sb[:, c], xt_sb[:, c, o:o + w],
                            start=(c == 0), stop=(c == NC - 1),
                        )
                for t, (o, w) in enumerate(subs):
                    hseg = h_sb[:, hc, o:o + w]
                    nc.scalar.activation(
                        hseg, pg[t][:, :w], mybir.ActivationFunctionType.Silu
                    )
                    nc.vector.tensor_mul(hseg, hseg, pu[t][:, :w])

            for oc in range(NC):
                wd_sb = wdp.tile([P, NH, P], BF, tag="wd")
                nc.sync.dma_start(wd_sb[:], wd_t[oc])
                py = [psp.tile([P, ps_w], F32, tag="ps", name=f"py{oc}_{t}")
                      for t in range(nt)]
                for hh in range(NH):
                    for t, (o, w) in enumerate(subs):
                        nc.tensor.matmul(
                            py[t][:, :w], wd_sb[:, hh], h_sb[:, hh, o:o + w],
                            start=(hh == 0), stop=(hh == NH - 1),
                        )
                y_sb = ypool.tile([P, cap], F32, tag="y")
                for t, (o, w) in enumerate(subs):
                    nc.vector.tensor_copy(y_sb[:, o:o + w], py[t][:, :w])
                nc.sync.dma_start(yt[oc * P:(oc + 1) * P, :], y_sb[:])

    nc.compile()
    return _patch_ldw_dedup(nc)


def _route(flat_x, Wr):
    """Host router: returns per-expert (token_idx, weight) with top-2 softmax."""
    n = flat_x.shape[0]
    logits = (flat_x @ Wr).astype(np.float32)
    ar = np.arange(n)
    i0 = logits.argmax(1)
    l0 = logits[ar, i0]
    masked = logits.copy()
    masked[ar, i0] = -np.inf
    i1 = masked.argmax(1)
    l1 = logits[ar, i1]
    # softmax over the two selected logits (l0 >= l1)
    e1 = np.exp((l1 - l0).astype(np.float32))
    w0 = 1.0 / (1.0 + e1)
    w1 = e1 / (1.0 + e1)
    experts = []
    for e in range(NUM_EXPERTS):
        m0 = i0 == e
        m1 = i1 == e
        idx = np.concatenate([ar[m0], ar[m1]])
        w = np.concatenate([w0[m0], w1[m1]]).astype(np.float32)
        experts.append((idx, w))
    return experts


def _in_maps(flat, Wg, Wu, Wd, experts, cap):
    maps = []
    for e in range(NUM_EXPERTS):
        idx, _ = experts[e]
        xt = np.zeros((D_MODEL, cap), dtype=BF16)
        xt[:, : len(idx)] = flat[idx].T.astype(BF16)
        wg_t = np.ascontiguousarray(
            Wg[e].reshape(NC, P, NH, P).transpose(2, 1, 0, 3)).astype(BF16)
        wu_t = np.ascontiguousarray(
            Wu[e].reshape(NC, P, NH, P).transpose(2, 1, 0, 3)).astype(BF16)
        wd_t = np.ascontiguousarray(
            Wd[e].reshape(NH, P, NC, P).transpose(2, 1, 0, 3)).astype(BF16)
        maps.append({"xt": xt, "wg_t": wg_t, "wu_t": wu_t, "wd_t": wd_t})
    return maps


def kernel(x, Wr, Wg, Wu, Wd):
    from concourse.bass_utils import run_bass_kernel_spmd

    B, T, C = x.shape
    x = np.asarray(x, dtype=np.float32)
    Wr = np.asarray(Wr, dtype=np.float32)
    Wg = np.asarray(Wg, dtype=np.float32)
    Wu = np.asarray(Wu, dtype=np.float32)
    Wd = np.asarray(Wd, dtype=np.float32)
    flat = x.reshape(-1, C)
    experts = _route(flat, Wr)

    n_max = max(len(idx) for idx, _ in experts)
    cap, _ = _token_subtiles(n_max)

    nc = _build_nc(cap)
    in_maps = _in_maps(flat, Wg, Wu, Wd, experts, cap)
    res = run_bass_kernel_spmd(nc, in_maps, core_ids=list(range(8)))

    out = np.zeros((B * T, C), dtype=np.float64)
    for e in range(NUM_EXPERTS):
        idx, w = experts[e]
        ye = res.results[e]["yt"].astype(np.float64).T[: len(idx)]
        out[idx] += (w.astype(np.float64) * DEPTH_SCALE)[:, None] * ye
    return out.astype(np.float32).reshape(B, T, C)


if __name__ == "__main__":
    import reference

    inputs = reference.setup_inputs()
    out = kernel(**{k: np.asarray(v) for k, v in inputs.items()})
    print("kernel output", out.shape, out.dtype)



# revision 13
# speedup vs baseline: 1.4730x; 1.4730x over previous
"""MoE layer (8 experts, top-2, SwiGLU) on 8 TRN2 NeuronCores.

Strategy: expert-parallel. The router (x @ Wr, top-2, softmax) runs on the
host — it is ~0.03% of the FLOPs. Tokens are dispatched per expert on the
host (the "all-to-all"), each core runs its expert's dense SwiGLU MLP over
its (capacity-padded) token batch, and the host applies the combine weights
(including DEPTH_SCALE) on the way back.

All matmul operands are bf16: on TRN2 the PE runs bf16 at the same
1 row/cycle as fp32r (measured), so bf16 keeps the same PE throughput while
halving DMA traffic and SBUF footprint. PSUM accumulation stays fp32; the
end-to-end relative error is ~4e-3. The halved footprint lets the whole
hidden dimension stay SBUF-resident: a single down-projection phase with
32-chunk accumulation chains and one output tensor.

Device layout (per core / expert e):
  xt   [C, CAP]            gathered tokens, transposed (feature-major), bf16
  wg_t [H/128,128,C/128,128]  Wg[e] pre-tiled so each stationary DMA is
  wu_t same                   contiguous, bf16
  wd_t [C/128,128,H/128,128]  Wd[e] pre-tiled, bf16
  yt   [C, CAP]            (silu(x@Wg) * (x@Wu)) @ Wd, transposed, fp32,
                           unscaled (host applies combine * DEPTH_SCALE)

The kernel is PE-bound: the ablated matmul-only instruction stream measures
the same wall time as the full kernel, i.e. weight streaming, SwiGLU and
output copies are fully hidden behind the matmul chain.

Optimization notes (second session, HW-measured via interleaved A/B on
same-REPEAT NEFFs — dispatch noise cancels):
- TimelineSim predicts 364us/exec (PE 97.2% busy at the 2.4GHz bf16
  roofline); HW measures ~435-440us, i.e. ~190ns per 364-wide matmul vs
  ~152ns predicted.
- A pure matmul-stream microbenchmark (one 3-tile chain set + copy
  consumer) runs AT roofline (~150ns/MM) on the same 8 cores, so the PE
  clock is 2.4GHz and per-MM LDWEIGHTS + semaphore increments are hidden
  by the PE queue in that shape.
- Adding a second interleaved chain set (the gate+up structure, 6 PSUM
  tiles/group) makes the added MMs cost ~195ns each REGARDLESS of: psum
  pool size (6 or 8 banks), chain length (4/8/16), forced pg-then-pu
  ordering (verified in the emitted stream), consumer type (act+mul vs
  plain copies), weight DMA streaming (none vs full), and LDW dedup /
  sem-inc stripping at the BIR level. Cause not identified; all cheap
  structural fixes were A/B-neutral on HW.
- fp8 (e4m3) DoubleRow would give ~1.8x PE throughput but numerically
  measures 4-6e-2 rel err on this MoE (vs the 2e-2 gate) for any matmul
  stage quantized, including mixed schemes; hi/lo-split compensation
  burns the 2x slot budget. Dead end.
- Expert imbalance is 1091 max vs 1024 mean (cap 1092): balance schemes
  (guest slots / H-split pairing) net only ~3% after padding. Not taken.
- _dedup_ldweights_json removes the 1536 redundant back-to-back LDWs
  walrus emits (one per MMUL; --enable-ldw-opt is hardcoded off and
  incompatible with bass BIR); validated correct, perf-neutral.
  _strip_incs_json (defer per-MM sem incs to chain ends + wait-value
  remap) is also validated+neutral, left off by default.
"""

import sys

if "/opt/trn_rl_repo" not in sys.path:
    sys.path.insert(0, "/opt/trn_rl_repo")

import numpy as np
import ml_dtypes

D_MODEL = 1024
HIDDEN = 4096
NUM_EXPERTS = 8
TOP_K = 2
DEPTH_SCALE = 1.0 / np.sqrt(12.0)

P = 128
NC = D_MODEL // P     # 8 column chunks of the model dim
NH = HIDDEN // P      # 32 chunks of the hidden dim

BF16 = ml_dtypes.bfloat16


def _dedup_ldweights_json(bir):
    """Remove redundant PE Ldweights: walrus pairs every MMUL with its own
    LDW (and --enable-ldw-opt is both hardcoded off and incompatible with
    bass BIR), so 3 consecutive matmuls on the same stationary tile reload
    it 3 times, ~27ns of serialized PE time each. Drop an Ldweights whose
    operands exactly match the previous Ldweights with no intervening
    PE-state-changing instruction; waits migrate to the next instruction."""
    import json as _json

    n_removed = 0
    for fn in bir["functions"]:
        for blk in fn["blocks"]:
            out = []
            last_key = None
            pending_waits = []
            for inst in blk["instructions"]:
                if inst.get("engine") != "PE":
                    out.append(inst)
                    continue
                op = inst.get("opcode")
                if op == "Ldweights":
                    key = _json.dumps(
                        [inst.get("ins"), inst.get("tile_position"),
                         inst.get("tile_size"), inst.get("perf_mode"),
                         inst.get("is_transpose")], sort_keys=True)
                    sync = inst.get("sync_info") or {}
                    if (key == last_key and not sync.get("on_update")):
                        n_removed += 1
                        pending_waits.extend(sync.get("on_wait") or [])
                        continue
                    last_key = key
                elif op != "Matmult":
                    last_key = None
                if pending_waits:
                    sync = inst.setdefault(
                        "sync_info", {"on_update": [], "on_wait": []})
                    sync.setdefault("on_wait", []).extend(pending_waits)
                    pending_waits = []
                out.append(inst)
            assert not pending_waits, "dangling waits from removed Ldweights"
            blk["instructions"] = out
    return n_removed


def _strip_incs_json(bir):
    """Drop the per-matmul semaphore increment on non-chain-final matmuls.

    Every matmul's then_inc is a serialized EVT_SEM register write (~26ns of
    PE engine time, measured); the Tile framework attaches one to every
    matmul, but downstream waits only ever reference counts reached at
    accumulation-chain ends (stop_tensor_calc matmuls). Keep those, drop the
    rest, and rewrite every wait threshold v on that semaphore to the rank of
    the first kept increment at-or-after the v-th original increment (waits
    can only become satisfied later, never earlier). Per-iteration
    sem-add/sub resets equal to the old total are rewritten to the new
    total."""
    n_removed = 0
    # sem ids incremented by PE Matmults
    sems = set()
    for fn in bir["functions"]:
        for blk in fn["blocks"]:
            for inst in blk["instructions"]:
                if inst.get("engine") == "PE" and \
                        inst.get("opcode") == "Matmult":
                    for u in (inst.get("sync_info") or {}).get(
                            "on_update", []) or []:
                        if u.get("update_mode") == "sem-inc":
                            sems.add(u["id"])
    for sid in sems:
        # collect inc sites per block
        blocks_with_incs = []
        for fn in bir["functions"]:
            for blk in fn["blocks"]:
                incs = []
                for inst in blk["instructions"]:
                    for u in (inst.get("sync_info") or {}).get(
                            "on_update", []) or []:
                        if u.get("id") == sid and \
                                u.get("update_mode") == "sem-inc":
                            assert u.get("update_value", 1) == 1
                            keep = (inst.get("opcode") != "Matmult" or
                                    bool(inst.get("stop_tensor_calc")))
                            incs.append((inst, keep))
                if incs:
                    blocks_with_incs.append((blk, incs))
        if len(blocks_with_incs) != 1:
            continue  # only handle the single-inc-block shape
        blk, incs = blocks_with_incs[0]
        if not incs[-1][1]:
            incs[-1] = (incs[-1][0], True)  # last inc must survive
        old_total = len(incs)
        # kept_rank_at_or_after[v-1] = new threshold for old threshold v
        new_total = sum(1 for _, k in incs if k)
        remap = [0] * (old_total + 1)
        rank = 0
        pending = []
        for i, (_, k) in enumerate(incs):
            pending.append(i + 1)
            if k:
                rank += 1
                for v in pending:
                    remap[v] = rank
                pending = []
        assert not pending
        n_exact = sum(1 for v in range(1, old_total + 1)
                      if remap[v] == sum(1 for _, k in incs[:v] if k))
        # strip non-kept incs
        for inst, k in incs:
            if not k:
                sync = inst["sync_info"]
                sync["on_update"] = [
                    u for u in sync["on_update"]
                    if not (u.get("id") == sid and
                            u.get("update_mode") == "sem-inc")]
                n_removed += 1
        # rewrite waits and add/sub totals everywhere
        for fn in bir["functions"]:
            for b in fn["blocks"]:
                for inst in b["instructions"]:
                    si = inst.get("sync_info") or {}
                    for w in si.get("on_wait", []) or []:
                        if w.get("id") == sid and \
                                w.get("wait_mode") == "sem-ge-imm":
                            v = w.get("wait_value")
                            assert 0 <= v <= old_total, (v, old_total)
                            w["wait_value"] = remap[v] if v else 0
                    for u in si.get("on_update", []) or []:
                        if u.get("id") == sid and u.get("update_mode") in (
                                "sem-add-imm", "sem-sub-imm"):
                            assert u.get("update_value") == old_total, u
                            u["update_value"] = new_total
    return n_removed


def _patch_bir(nc, dedup_ldw=True, strip_incs=False):
    import json as _json

    orig = getattr(nc, "_orig_to_json_bytes", None) or nc.to_json_bytes
    nc._orig_to_json_bytes = orig

    def patched():
        bir = _json.loads(orig())
        if dedup_ldw:
            _dedup_ldweights_json(bir)
        if strip_incs:
            _strip_incs_json(bir)
        return _json.dumps(bir).encode()

    nc.to_json_bytes = patched
    return nc


def _token_subtiles(cap):
    """Pad cap so it splits into equal even-width sub-tiles in [256, 512]
    (full PE rate needs a wide moving free dim; the PSUM bank caps fp32
    matmul width at 512). Returns (padded_cap, subs)."""
    cap = max(256, cap)
    nt = max(1, -(-cap // 512))
    step = 2 * nt
    cap = -(-cap // step) * step
    size = cap // nt
    assert 256 <= size <= 512 and size % 2 == 0, (cap, size)
    return cap, [(i * size, size) for i in range(nt)]


def _build_nc(cap, repeat=1, wgu_bufs=3, wd_bufs=2, y_bufs=2,
              order="default"):
    import concourse.mybir as mybir
    import concourse.tile as tile
    from concourse import bacc

    F32 = mybir.dt.float32
    BF = mybir.dt.bfloat16
    cap2, subs = _token_subtiles(cap)
    assert cap2 == cap, (cap2, cap)
    nt = len(subs)
    ps_w = max(s for _, s in subs)

    nc = bacc.Bacc("TRN2", target_bir_lowering=False, debug=False, num_devices=8)
    xt = nc.dram_tensor("xt", [D_MODEL, cap], BF, kind="ExternalInput").ap()
    wg_t = nc.dram_tensor("wg_t", [NH, P, NC, P], BF, kind="ExternalInput").ap()
    wu_t = nc.dram_tensor("wu_t", [NH, P, NC, P], BF, kind="ExternalInput").ap()
    wd_t = nc.dram_tensor("wd_t", [NC, P, NH, P], BF, kind="ExternalInput").ap()
    yt = nc.dram_tensor("yt", [D_MODEL, cap], F32, kind="ExternalOutput").ap()

    from contextlib import ExitStack

    with tile.TileContext(nc) as tc:
        rep = ExitStack()
        if repeat > 1:
            rep.enter_context(tc.For_i(0, repeat, 1))
        with (
            rep,
            tc.tile_pool(name="xpool", bufs=1) as xpool,
            tc.tile_pool(name="hpool", bufs=1) as hpool,
            tc.tile_pool(name="wg", bufs=wgu_bufs) as wgp,
            tc.tile_pool(name="wu", bufs=wgu_bufs) as wup,
            tc.tile_pool(name="wd", bufs=wd_bufs) as wdp,
            tc.tile_pool(name="ypool", bufs=y_bufs) as ypool,
            tc.tile_pool(name="psum", bufs=8, space="PSUM") as psp,
        ):
            # token activations, split per c-chunk so the first gate chain
            # only waits on chunk 0; the first weight tiles are queued right
            # after chunk 0 so the PE can start before the full xt lands
            xt_sb = xpool.tile([P, NC, cap], BF)
            xt_r = xt.rearrange("(o p) n -> p o n", p=P)
            nc.sync.dma_start(xt_sb[:, 0], xt_r[:, 0])
            wg0_sb = wgp.tile([P, NC, P], BF, tag="wg")
            wu0_sb = wup.tile([P, NC, P], BF, tag="wu")
            nc.sync.dma_start(wg0_sb[:], wg_t[0])
            nc.sync.dma_start(wu0_sb[:], wu_t[0])
            for c in range(1, NC):
                nc.sync.dma_start(xt_sb[:, c], xt_r[:, c])

            from contextlib import nullcontext

            T_ms = 0.0036  # one gate (or up) half-group of PE time
            h_sb = hpool.tile([P, NH, cap], BF, tag="h")
            for hc in range(NH):
                if hc == 0:
                    wg_sb, wu_sb = wg0_sb, wu0_sb
                else:
                    wg_sb = wgp.tile([P, NC, P], BF, tag="wg")
                    wu_sb = wup.tile([P, NC, P], BF, tag="wu")
                    nc.sync.dma_start(wg_sb[:], wg_t[hc])
                    nc.sync.dma_start(wu_sb[:], wu_t[hc])

                pg = [psp.tile([P, ps_w], F32, tag="ps", name=f"pg{hc}_{t}")
                      for t in range(nt)]
                pu = [psp.tile([P, ps_w], F32, tag="ps", name=f"pu{hc}_{t}")
                      for t in range(nt)]
                # schedule gate chains strictly before up chains (wait-ts
                # floors): chains then finish mid-group, so the silu/mul
                # consumers drain PSUM banks during the second half instead
                # of piling up at the group boundary and stalling the PE
                with (tc.tile_wait_until(2 * hc * T_ms)
                      if order == "split" else nullcontext()):
                    for c in range(NC):
                        for t, (o, w) in enumerate(subs):
                            nc.tensor.matmul(
                                pg[t][:, :w], wg_sb[:, c],
                                xt_sb[:, c, o:o + w],
                                start=(c == 0), stop=(c == NC - 1),
                            )
                with (tc.tile_wait_until((2 * hc + 1) * T_ms)
                      if order == "split" else nullcontext()):
                    for c in range(NC):
                        for t, (o, w) in enumerate(subs):
                            nc.tensor.matmul(
                                pu[t][:, :w], wu_sb[:, c],
                                xt_sb[:, c, o:o + w],
                                start=(c == 0), stop=(c == NC - 1),
                            )
                for t, (o, w) in enumerate(subs):
                    hseg = h_sb[:, hc, o:o + w]
                    nc.scalar.activation(
                        hseg, pg[t][:, :w], mybir.ActivationFunctionType.Silu
                    )
                    nc.vector.tensor_mul(hseg, hseg, pu[t][:, :w])

            for oc in range(NC):
                wd_sb = wdp.tile([P, NH, P], BF, tag="wd")
                nc.sync.dma_start(wd_sb[:], wd_t[oc])
                py = [psp.tile([P, ps_w], F32, tag="ps", name=f"py{oc}_{t}")
                      for t in range(nt)]
                for hh in range(NH):
                    for t, (o, w) in enumerate(subs):
                        nc.tensor.matmul(
                            py[t][:, :w], wd_sb[:, hh], h_sb[:, hh, o:o + w],
                            start=(hh == 0), stop=(hh == NH - 1),
                        )
                y_sb = ypool.tile([P, cap], F32, tag="y")
                for t, (o, w) in enumerate(subs):
                    nc.vector.tensor_copy(y_sb[:, o:o + w], py[t][:, :w])
                nc.sync.dma_start(yt[oc * P:(oc + 1) * P, :], y_sb[:])

    nc.compile()
    return _patch_bir(nc)


def _route(flat_x, Wr):
    """Host router: returns per-expert (token_idx, weight) with top-2 softmax."""
    n = flat_x.shape[0]
    logits = (flat_x @ Wr).astype(np.float32)
    ar = np.arange(n)
    i0 = logits.argmax(1)
    l0 = logits[ar, i0]
    masked = logits.copy()
    masked[ar, i0] = -np.inf
    i1 = masked.argmax(1)
    l1 = logits[ar, i1]
    # softmax over the two selected logits (l0 >= l1)
    e1 = np.exp((l1 - l0).astype(np.float32))
    w0 = 1.0 / (1.0 + e1)
    w1 = e1 / (1.0 + e1)
    experts = []
    for e in range(NUM_EXPERTS):
        m0 = i0 == e
        m1 = i1 == e
        idx = np.concatenate([ar[m0], ar[m1]])
        w = np.concatenate([w0[m0], w1[m1]]).astype(np.float32)
        experts.append((idx, w))
    return experts


def _in_maps(flat, Wg, Wu, Wd, experts, cap):
    maps = []
    for e in range(NUM_EXPERTS):
        idx, _ = experts[e]
        xt = np.zeros((D_MODEL, cap), dtype=BF16)
        xt[:, : len(idx)] = flat[idx].T.astype(BF16)
        wg_t = np.ascontiguousarray(
            Wg[e].reshape(NC, P, NH, P).transpose(2, 1, 0, 3)).astype(BF16)
        wu_t = np.ascontiguousarray(
            Wu[e].reshape(NC, P, NH, P).transpose(2, 1, 0, 3)).astype(BF16)
        wd_t = np.ascontiguousarray(
            Wd[e].reshape(NH, P, NC, P).transpose(2, 1, 0, 3)).astype(BF16)
        maps.append({"xt": xt, "wg_t": wg_t, "wu_t": wu_t, "wd_t": wd_t})
    return maps


def kernel(x, Wr, Wg, Wu, Wd):
    from concourse.bass_utils import run_bass_kernel_spmd

    B, T, C = x.shape
    x = np.asarray(x, dtype=np.float32)
    Wr = np.asarray(Wr, dtype=np.float32)
    Wg = np.asarray(Wg, dtype=np.float32)
    Wu = np.asarray(Wu, dtype=np.float32)
    Wd = np.asarray(Wd, dtype=np.float32)
    flat = x.reshape(-1, C)
    experts = _route(flat, Wr)

    n_max = max(len(idx) for idx, _ in experts)
    cap, _ = _token_subtiles(n_max)

    nc = _build_nc(cap)
    in_maps = _in_maps(flat, Wg, Wu, Wd, experts, cap)
    res = run_bass_kernel_spmd(nc, in_maps, core_ids=list(range(8)))

    out = np.zeros((B * T, C), dtype=np.float64)
    for e in range(NUM_EXPERTS):
        idx, w = experts[e]
        ye = res.results[e]["yt"].astype(np.float64).T[: len(idx)]
        out[idx] += (w.astype(np.float64) * DEPTH_SCALE)[:, None] * ye
    return out.astype(np.float32).reshape(B, T, C)


if __name__ == "__main__":
    import reference

    inputs = reference.setup_inputs()
    out = kernel(**{k: np.asarray(v) for k, v in inputs.items()})
    print("kernel output", out.shape, out.dtype)

